# revision 27
# baseline (speedup 1.0000x reference)
"""Trainium2 Bass kernel for nn_AttentionNet (BiDAF-style attention + 3 BiLSTM).

Data-parallel over batch B=8 across 8 NeuronCores; one batch element per core.
All tensors live feature-on-partition / T-on-free, so no transposes are needed
except tiny PE transposes inside the attention softmax.

LSTM recurrence: gates-on-partition layout. Gate vector (800) is permuted and
padded to 8 chunks of 128: [i0 i1 f0 f1 o0 o1 g0 g1] (k = unit index chunks
0:128 / 128:200+pad). Per tick and direction: 16 weight-stationary LDW+MM
pairs (k in {0,1} x m in 0..7), then sigmoid/tanh + cell update on 128
partitions. Padding rows self-clean (h_pad stays 0).
"""
import os
import sys
import numpy as np
import ml_dtypes

os.environ.setdefault("JAX_COMPILATION_CACHE_DIR", "/tmp/jax_neff_cache")
os.environ.setdefault("JAX_PERSISTENT_CACHE_MIN_COMPILE_TIME_SECS", "1")
os.environ.setdefault("JAX_PERSISTENT_CACHE_MIN_ENTRY_SIZE_BYTES", "0")
sys.path.insert(0, "/opt/trn_rl_repo")
from concourse import bacc, tile, mybir  # noqa: E402
from concourse.bass_utils import run_bass_kernel_spmd  # noqa: E402

dt = mybir.dt
AF = mybir.ActivationFunctionType
ALU = mybir.AluOpType

B, T, J = 8, 512, 64
H = 200
D2 = 400
NDIR = 2
BF = ml_dtypes.bfloat16

# gate permutation: old rows [i(200) f(200) g(200) o(200)] -> 8 chunks of 128
PERM_SRCS = [(0, 0, 128), (128, 128, 72), (200, 256, 128), (328, 384, 72),
             (600, 512, 128), (728, 640, 72), (400, 768, 128), (528, 896, 72)]
MBOUNDS = [(0, 128), (128, 200), (200, 328), (328, 400)]  # 400-dim k-chunk bounds


def perm_pad(vec800_last):
    out = np.zeros(vec800_last.shape[:-1] + (1024,), vec800_last.dtype)
    for so, do, n in PERM_SRCS:
        out[..., do:do + n] = vec800_last[..., so:so + n]
    return out


_CACHED = {}


def _build(t_len):
    nc = bacc.Bacc("TRN2", target_bir_lowering=False, debug=False, num_devices=8)
    NT = t_len // 128
    TPL = t_len + 4  # h column count (cols 0 and t_len+1 zero; +2 pad for 4B align)

    d = {}

    def dram(name, shape, dty=dt.float32, out=False):
        d[name] = nc.declare_dram_parameter(name, list(shape), dty, isOutput=out)
        return d[name]

    dram("ecT", [100, 4, t_len])          # ec transposed, feature chunks of 100
    dram("ec", [128, NT, D2])             # ec, T chunks of 128
    dram("eqT", [100, 4, J])
    dram("eq", [J, D2])
    dram("w1", [100, 4, 1])
    dram("w3", [100, 4, 1])
    dram("w2rep", [100, 4, 128])
    dram("ident", [128, 128])
    dram("ones", [128, 128])
    dram("wih0", [100, NDIR, 16, 1024], dt.bfloat16)
    dram("wih1", [128, NDIR, 4, 1024], dt.float8e3)
    dram("wih2", [128, NDIR, 4, 1024], dt.float8e3)
    for li in range(3):
        dram(f"whh{li}", [128, NDIR, 2, 1024], dt.float8e3)
        dram(f"b{li}", [128, NDIR, 8])
    dram("wp1g", [100, 16, 1])
    dram("wp1m", [128, 4, 1])
    dram("wp2g", [100, 16, 1])
    dram("wp2m", [128, 4, 1])
    dram("p1out", [128, NT], out=True)
    dram("p2out", [128, NT], out=True)

    with tile.TileContext(nc) as tc:
        with (
            tc.tile_pool(name="big", bufs=1) as big,
            tc.tile_pool(name="work", bufs=3) as work,
        ):
            # ---------- load inputs ----------
            ecT = big.tile([100, 4, t_len], dt.float32, name="ecT")
            ec = big.tile([128, NT, D2], dt.float32, name="ec")
            eqT = big.tile([100, 4, J], dt.float32, name="eqT")
            eq = big.tile([J, D2], dt.float32, name="eq")
            w1 = big.tile([100, 4, 1], dt.float32, name="w1")
            w3 = big.tile([100, 4, 1], dt.float32, name="w3")
            w2rep = big.tile([100, 4, 128], dt.float32, name="w2rep")
            ident = big.tile([128, 128], dt.float32, name="ident")
            ones = big.tile([128, 128], dt.float32, name="ones")
            for nm, tl in [("ecT", ecT), ("ec", ec), ("eqT", eqT), ("eq", eq),
                           ("w1", w1), ("w3", w3), ("w2rep", w2rep),
                           ("ident", ident), ("ones", ones)]:
                nc.sync.dma_start(out=tl[:], in_=d[nm].ap())

            ecTb = big.tile([100, 4, t_len], dt.bfloat16, name="ecTb")
            ecb = big.tile([128, NT, D2], dt.bfloat16, name="ecb")
            eqTb = big.tile([100, 4, J], dt.bfloat16, name="eqTb")
            eqb = big.tile([J, D2], dt.bfloat16, name="eqb")
            identb = big.tile([128, 128], dt.bfloat16, name="identb")
            w2repb = big.tile([100, 4, 128], dt.bfloat16, name="w2repb")
            ecw3Tb = big.tile([100, 4, t_len], dt.bfloat16, name="ecw3Tb")
            nc.any.tensor_copy(ecTb[:], ecT[:])
            nc.any.tensor_copy(ecb[:], ec[:])
            nc.any.tensor_copy(eqTb[:], eqT[:])
            nc.any.tensor_copy(eqb[:], eq[:])
            nc.any.tensor_copy(identb[:], ident[:])
            nc.any.tensor_copy(w2repb[:], w2rep[:])
            for fc in range(4):
                nc.vector.tensor_scalar_mul(ecw3Tb[:, fc], ecT[:, fc], w3[:, fc])

            # ---------- attention ----------
            Ptb = []
            mrow = []
            eb_all = big.tile([128, NT], dt.float32, name="eb_all")
            with tc.tile_pool(name="psA", bufs=1, space="PSUM") as ppa:
                for tch in range(NT):
                    sl = slice(tch * 128, (tch + 1) * 128)
                    ps_s = ppa.tile([128, J], dt.float32, name="ps_s", tag="ps_s")
                    for fc in range(4):
                        nc.tensor.matmul(ps_s[:], ecw3Tb[:, fc, sl], eqTb[:, fc],
                                         start=(fc == 0), stop=False)
                    for fc in range(4):
                        nc.tensor.matmul(ps_s[:], w2repb[:, fc], eqTb[:, fc],
                                         start=False, stop=(fc == 3))
                    ps_w1 = ppa.tile([128, 1], dt.float32, name="ps_w1", tag="ps_w1")
                    for fc in range(4):
                        nc.tensor.matmul(ps_w1[:], ecT[:, fc, sl], w1[:, fc],
                                         start=(fc == 0), stop=(fc == 3))
                    negr = work.tile([128, 1], dt.float32, name="negr", tag="negr")
                    nc.vector.tensor_reduce(negr[:], ps_s[:], mybir.AxisListType.X,
                                            ALU.max, negate=True)
                    e_t = work.tile([128, J], dt.float32, name="e_t", tag="e_t")
                    zsum = work.tile([128, 1], dt.float32, name="zsum", tag="zsum")
                    nc.scalar.activation(e_t[:], ps_s[:], AF.Exp, bias=negr[:],
                                         accum_out=zsum[:])
                    invz = work.tile([128, 1], dt.float32, name="invz", tag="invz")
                    nc.vector.reciprocal(invz[:], zsum[:])
                    pnorm = work.tile([128, J], dt.bfloat16, name="pnorm", tag="pnorm")
                    nc.vector.tensor_scalar_mul(pnorm[:], e_t[:], invz[:])
                    mr = work.tile([128, 1], dt.float32, name="mr", tag=f"mr{tch}")
                    nc.vector.scalar_tensor_tensor(mr[:], negr[:], -1.0, ps_w1[:],
                                                   ALU.mult, ALU.add)
                    mrow.append(mr)
                    ps_pt = ppa.tile([J, 128], dt.bfloat16, name="ps_pt", tag="ps_pt")
                    nc.tensor.transpose(ps_pt[:], pnorm[:], identb[:])
                    ptb = big.tile([J, 128], dt.bfloat16, name=f"ptb{tch}")
                    nc.any.tensor_copy(ptb[:], ps_pt[:])
                    Ptb.append(ptb)

                for tch in range(NT):
                    nc.scalar.activation(eb_all[:, tch:tch + 1], mrow[tch][:], AF.Exp)
                ps_zb = ppa.tile([1, NT], dt.float32, name="ps_zb", tag="ps_zb")
                nc.tensor.matmul(ps_zb[:], ones[:, 0:1], eb_all[:], start=True, stop=True)
                zb = work.tile([1, 1], dt.float32, name="zb")
                nc.vector.tensor_reduce(zb[:], ps_zb[:], mybir.AxisListType.X, ALU.add)
                invzb = work.tile([1, 1], dt.float32, name="invzb")
                nc.vector.reciprocal(invzb[:], zb[:])
                ps_izb = ppa.tile([128, 1], dt.float32, name="ps_izb", tag="ps_izb")
                nc.tensor.matmul(ps_izb[:], ones[0:1, :], invzb[:], start=True, stop=True)
                izb = work.tile([128, 1], dt.float32, name="izb")
                nc.any.tensor_copy(izb[:], ps_izb[:])
                ebn = big.tile([128, NT], dt.float32, name="ebn")
                nc.vector.tensor_scalar_mul(ebn[:], eb_all[:], izb[:])

                q2cT = big.tile([100, 4, 1], dt.float32, name="q2cT")
                for fc in range(4):
                    ps_q = ppa.tile([100, 1], dt.float32, name="ps_q", tag="ps_q")
                    for tch in range(NT):
                        nc.tensor.matmul(ps_q[:], ec[:, tch, fc * 100:(fc + 1) * 100],
                                         ebn[:, tch:tch + 1],
                                         start=(tch == 0), stop=(tch == NT - 1))
                    nc.any.tensor_copy(q2cT[:, fc], ps_q[:])

                Gtiles = [ecTb[:, fc] for fc in range(4)]
                g_c2q, g_pc, g_pq = [], [], []
                for fc in range(4):
                    ps_c = ppa.tile([100, t_len], dt.float32, name="ps_c", tag="ps_c")
                    for tch in range(NT):
                        nc.tensor.matmul(ps_c[:, tch * 128:(tch + 1) * 128],
                                         eqb[:, fc * 100:(fc + 1) * 100], Ptb[tch][:],
                                         start=True, stop=True)
                    c2q_f = big.tile([100, t_len], dt.bfloat16, name=f"c2q{fc}")
                    nc.any.tensor_copy(c2q_f[:], ps_c[:])
                    g2 = big.tile([100, t_len], dt.bfloat16, name=f"g2_{fc}")
                    nc.vector.scalar_tensor_tensor(g2[:], ps_c[:], 1.0, ecT[:, fc],
                                                   ALU.mult, ALU.mult)
                    g3 = big.tile([100, t_len], dt.bfloat16, name=f"g3_{fc}")
                    nc.vector.tensor_scalar_mul(g3[:], ecT[:, fc], q2cT[:, fc])
                    g_c2q.append(c2q_f)
                    g_pc.append(g2)
                    g_pq.append(g3)
                Gtiles += [x[:] for x in g_c2q] + [x[:] for x in g_pc] + [x[:] for x in g_pq]

            # ---------- LSTM stack ----------
            def lstm_layer(li, in_tiles, in_kdims, nk_wih, kpart):
                wih_dt = dt.bfloat16 if li == 0 else dt.float8e3
                whh = [big.tile([128, 2, 1024], dt.float8e3, name=f"whh{li}_{dd}", tag=f"whhS_{dd}")
                       for dd in range(NDIR)]
                bvec = [big.tile([128, 8], dt.float32, name=f"b{li}_{dd}", tag=f"bS_{dd}")
                        for dd in range(NDIR)]
                wih = [big.tile([kpart, nk_wih, 1024], wih_dt, name=f"wih{li}_{dd}", tag=f"wihS_{dd}")
                       for dd in range(NDIR)]
                for dd in range(NDIR):
                    nc.sync.dma_start(out=whh[dd][:], in_=d[f"whh{li}"].ap()[:, dd])
                    nc.sync.dma_start(out=bvec[dd][:], in_=d[f"b{li}"].ap()[:, dd])
                    nc.sync.dma_start(out=wih[dd][:], in_=d[f"wih{li}"].ap()[:, dd])

                pre = [big.tile([128, t_len, 8], dt.bfloat16, name=f"pre{li}_{dd}", tag=f"preS_{dd}")
                       for dd in range(NDIR)]
                with tc.tile_pool(name=f"psP{li}", bufs=2, space="PSUM") as ppp:
                    for dd in range(NDIR):
                        for m in range(8):
                            ps_p = ppp.tile([128, t_len], dt.float32, name="ps_p", tag="ps_p")
                            for ki, (ap_k, kdim) in enumerate(zip(in_tiles, in_kdims)):
                                nc.tensor.matmul(
                                    ps_p[:],
                                    wih[dd][0:kdim, ki, m * 128:(m + 1) * 128],
                                    ap_k[0:kdim, :],
                                    start=(ki == 0), stop=(ki == len(in_tiles) - 1))
                            nc.vector.tensor_scalar_add(pre[dd][:, :, m], ps_p[:],
                                                        bvec[dd][:, m:m + 1])

                # x8-scaled fp8 h, flat [units, time*2] with col t*2+half: the
                # only h storage. Contiguous [128, 2] per-tick writes; later
                # layers/heads read stride-2 time slices.
                h8 = [big.tile([128, TPL * 2], dt.float8e3, name=f"h8{li}_{dd}", tag=f"h8S{li}_{dd}")
                      for dd in range(NDIR)]
                for dd in range(NDIR):
                    nc.any.memzero(h8[dd][:])
                # c for both dirs: cols [f_lo, f_hi, b_lo, b_hi]
                c_prev = work.tile([128, 4], dt.float32, name="c0", tag=f"c{li}x1")
                nc.any.memzero(c_prev[:])

                with tc.tile_pool(name=f"psR{li}", bufs=2, space="PSUM") as ppr:
                    for tt in range(t_len):
                        t_f, t_b = tt, t_len - 1 - tt
                        rd = (t_f * 2, (t_b + 2) * 2)          # h8 read col base per dir
                        wr = ((t_f + 1) * 2, (t_b + 1) * 2)    # h8 write col base per dir
                        # one PSUM tile [128, dir, chunk] for both directions
                        ps = ppr.tile([128, 2, 8], dt.float32, name="ps", tag="psB")
                        if tt < 2:
                            # prime this PSUM buffer's has_written bits with a
                            # real accumulation group
                            nc.tensor.matmul(ps[:, 0, :], identb[:], pre[0][:, t_f, :],
                                             start=True, stop=False)
                            nc.tensor.matmul(ps[:, 1, :], identb[:], pre[1][:, t_b, :],
                                             start=False, stop=False)
                        else:
                            # pre lands in PSUM via DVE; matmuls accumulate onto
                            # it (bank bits stay "written" since priming)
                            nc.vector.tensor_copy(ps[:, 0, :], pre[0][:, t_f, :])
                            nc.vector.tensor_copy(ps[:, 1, :], pre[1][:, t_b, :])
                        for dd in range(NDIR):
                            rhs0 = h8[dd][:, rd[dd]:rd[dd] + 1]
                            rhs1 = h8[dd][:, rd[dd] + 1:rd[dd] + 2]
                            last = (tt < 2 and dd == 1)
                            skip = tt >= 2
                            for m in range(8):
                                nc.tensor.matmul(ps[:, dd, m:m + 1],
                                                 whh[dd][:, 0, m * 128:(m + 1) * 128],
                                                 rhs0, start=False, stop=False,
                                                 skip_group_check=skip)
                                nc.tensor.matmul(ps[:, dd, m:m + 1],
                                                 whh[dd][:, 1, m * 128:(m + 1) * 128],
                                                 rhs1, start=False,
                                                 stop=(last and m == 7),
                                                 skip_group_check=skip)
                        # s8 = sigmoid of all gate chunks, both dirs at once;
                        # preacts carry x256 (fp8 weight/h scales), g chunks x2
                        s8 = work.tile([128, 2, 8], dt.float32, name="s8", tag=f"s{li}")
                        nc.scalar.activation(s8[:], ps[:], AF.Sigmoid, scale=0.00390625)
                        # tanh(g) = 2*sigmoid(2g) - 1 : u = si*tg = 2*sg*si - si
                        t1 = work.tile([128, 4], dt.float32, name="t1", tag=f"t1{li}")
                        u = work.tile([128, 4], dt.float32, name="u", tag=f"u{li}")
                        v = work.tile([128, 4], dt.float32, name="v", tag=f"v{li}")
                        c_new = work.tile([128, 4], dt.float32, name="cn", tag=f"c{li}x{tt % 2}")
                        nc.vector.scalar_tensor_tensor(t1[:], s8[:, :, 6:8], 2.0, s8[:, :, 0:2],
                                                       ALU.mult, ALU.mult)
                        nc.vector.scalar_tensor_tensor(u[:], t1[:], 1.0, s8[:, :, 0:2],
                                                       ALU.mult, ALU.subtract)
                        nc.vector.scalar_tensor_tensor(v[:], c_prev[:], 0.0, s8[:, :, 2:4],
                                                       ALU.add, ALU.mult)
                        nc.vector.scalar_tensor_tensor(c_new[:], u[:], 0.0, v[:],
                                                       ALU.add, ALU.add)
                        # h8 = 8 * tanh(c) * so
                        th = work.tile([128, 4], dt.float32, name="th", tag=f"th{li}")
                        nc.scalar.activation(th[:], c_new[:], AF.Tanh)
                        nc.vector.scalar_tensor_tensor(h8[0][:, wr[0]:wr[0] + 2],
                                                       th[:, 0:2], 8.0, s8[:, 0, 4:6],
                                                       ALU.mult, ALU.mult)
                        nc.vector.scalar_tensor_tensor(h8[1][:, wr[1]:wr[1] + 2],
                                                       th[:, 2:4], 8.0, s8[:, 1, 4:6],
                                                       ALU.mult, ALU.mult)
                        c_prev = c_new
                return h8

            def h8_tiles(h8):
                # [fwd_lo, fwd_hi, bwd_lo, bwd_hi] over the valid time range
                a, b_ = 2, 2 + 2 * t_len
                return [h8[0][:, a:b_:2], h8[0][0:72, a + 1:b_:2],
                        h8[1][:, a:b_:2], h8[1][0:72, a + 1:b_:2]]

            h0 = lstm_layer(0, Gtiles, [100] * 16, 16, 100)
            mk = [128, 72, 128, 72]
            m0_tiles = h8_tiles(h0)
            h1 = lstm_layer(1, m0_tiles, mk, 4, 128)
            m1_tiles = h8_tiles(h1)
            h2 = lstm_layer(2, m1_tiles, mk, 4, 128)
            m2_tiles = h8_tiles(h2)

            # ---------- heads ----------
            # logits carry a x128 scale: G-part weights are x128 (bf16),
            # M-part weights are x16 (fp8e3) against the x8-scaled h8; the
            # exp undoes it with scale=1/128.
            wpb = {}
            for nm, shp in [("wp1g", [100, 16, 1]), ("wp1m", [128, 4, 1]),
                            ("wp2g", [100, 16, 1]), ("wp2m", [128, 4, 1])]:
                tl = work.tile(shp, dt.float32, name=nm, tag=nm)
                nc.sync.dma_start(out=tl[:], in_=d[nm].ap())
                wdt = dt.float8e3 if nm.endswith("m") else dt.bfloat16
                tb = work.tile(shp, wdt, name=nm + "b", tag=nm + "b")
                nc.any.tensor_copy(tb[:], tl[:])
                wpb[nm] = tb

            with tc.tile_pool(name="psH", bufs=2, space="PSUM") as pph:
                def head(gname, mname, m_tiles, out_name):
                    e_all = work.tile([128, NT], dt.float32, name=f"e_{gname}")
                    for tch2 in range(NT):
                        sl = slice(tch2 * 128, (tch2 + 1) * 128)
                        ps_l = pph.tile([128, 1], dt.float32, name="ps_l", tag="ps_l")
                        for gi in range(16):
                            nc.tensor.matmul(ps_l[:], Gtiles[gi][:, sl], wpb[gname][:, gi],
                                             start=(gi == 0), stop=False)
                        for ki in range(4):
                            nc.tensor.matmul(ps_l[:], m_tiles[ki][0:mk[ki], sl],
                                             wpb[mname][0:mk[ki], ki],
                                             start=False, stop=(ki == 3))
                        nc.scalar.activation(e_all[:, tch2:tch2 + 1], ps_l[:], AF.Exp,
                                             scale=0.0078125)
                    ps_z = pph.tile([1, NT], dt.float32, name="ps_z", tag="ps_z")
                    nc.tensor.matmul(ps_z[:], ones[:, 0:1], e_all[:], start=True, stop=True)
                    z = work.tile([1, 1], dt.float32, name=f"z_{gname}")
                    nc.vector.tensor_reduce(z[:], ps_z[:], mybir.AxisListType.X, ALU.add)
                    iz = work.tile([1, 1], dt.float32, name=f"iz_{gname}")
                    nc.vector.reciprocal(iz[:], z[:])
                    ps_i = pph.tile([128, 1], dt.float32, name="ps_i", tag="ps_i")
                    nc.tensor.matmul(ps_i[:], ones[0:1, :], iz[:], start=True, stop=True)
                    izr = work.tile([128, 1], dt.float32, name=f"izr_{gname}")
                    nc.any.tensor_copy(izr[:], ps_i[:])
                    pout = work.tile([128, NT], dt.float32, name=f"pout_{gname}")
                    nc.vector.tensor_scalar_mul(pout[:], e_all[:], izr[:])
                    nc.sync.dma_start(out=d[out_name].ap(), in_=pout[:])

                head("wp1g", "wp1m", m1_tiles, "p1out")
                head("wp2g", "wp2m", m2_tiles, "p2out")

    nc.compile()
    return nc


def _prep_shared(w_s, Wih0, Whh0, b0, Wih1, Whh1, b1, w_p1, Wih2, Whh2, b2, w_p2, b_p2):
    sh = {}
    w1v, w2v, w3v = w_s[:D2], w_s[D2:2 * D2], w_s[2 * D2:]
    sh["w1"] = w1v.reshape(4, 100).T.reshape(100, 4, 1).astype(np.float32).copy()
    sh["w3"] = w3v.reshape(4, 100).T.reshape(100, 4, 1).astype(np.float32).copy()
    sh["w2rep"] = np.repeat(w2v.reshape(4, 100).T.reshape(100, 4, 1), 128, axis=2).astype(np.float32)
    sh["ident"] = np.eye(128, dtype=np.float32)
    sh["ones"] = np.ones((128, 128), np.float32)

    # whh carries x32 (fits fp8e3 normal range), h8 carries x8, so the full
    # recurrent product is x256; wih/b are x256 so preactivations match, and
    # the scan's sigmoid uses scale=1/256 to undo it.
    QW, QP = 32.0, 256.0

    def whh_pack(Whh):
        WP = perm_pad(np.swapaxes(Whh, 1, 2))  # [2, 200, 1024]
        WP[..., 768:1024] *= 2.0
        out = np.zeros((128, NDIR, 2, 1024), np.float32)
        out[:, :, 0] = WP[:, 0:128].transpose(1, 0, 2)
        out[0:72, :, 1] = WP[:, 128:200].transpose(1, 0, 2)
        return np.clip(out * QW, -15.5, 15.5).astype(ml_dtypes.float8_e3m4)

    def bias_pack(b):
        bP = perm_pad(b)  # [2, 1024]
        bP[..., 768:1024] *= 2.0
        return (bP.reshape(NDIR, 8, 128).transpose(2, 0, 1) * QP).copy().astype(np.float32)

    def wih_pack(Wih, nk, kdim, fp8=False):
        WP = perm_pad(np.swapaxes(Wih, 1, 2))  # [2, IN, 1024]
        WP[..., 768:1024] *= 2.0
        out = np.zeros((kdim, NDIR, nk, 1024), np.float32)
        if kdim == 100:
            for k in range(nk):
                out[:, :, k] = WP[:, k * 100:(k + 1) * 100].transpose(1, 0, 2)
        else:
            for k, (a, b_) in enumerate(MBOUNDS):
                out[0:b_ - a, :, k] = WP[:, a:b_].transpose(1, 0, 2)
        if fp8:  # consumed against x8-scaled h8 -> x32 weight keeps x256 total
            return np.clip(out * QW, -15.5, 15.5).astype(ml_dtypes.float8_e3m4)
        return (out * QP).astype(BF)

    sh["wih0"] = wih_pack(Wih0, 16, 100)
    sh["wih1"] = wih_pack(Wih1, 4, 128, fp8=True)
    sh["wih2"] = wih_pack(Wih2, 4, 128, fp8=True)
    sh["whh0"] = whh_pack(Whh0)
    sh["whh1"] = whh_pack(Whh1)
    sh["whh2"] = whh_pack(Whh2)
    sh["b0"] = bias_pack(b0)
    sh["b1"] = bias_pack(b1)
    sh["b2"] = bias_pack(b2)

    def mpack(wm):
        out = np.zeros((128, 4, 1), np.float32)
        for k, (a, b_) in enumerate(MBOUNDS):
            out[0:b_ - a, k, 0] = wm[a:b_]
        return out

    # head logit scale: G-part x128, M-part x16 (reads x8-scaled h8)
    sh["wp1g"] = (w_p1[:1600].reshape(16, 100).T.reshape(100, 16, 1) * 128.0).astype(np.float32).copy()
    sh["wp1m"] = np.clip(mpack(w_p1[1600:]) * 16.0, -15.5, 15.5)
    sh["wp2g"] = (w_p2[:1600].reshape(16, 100).T.reshape(100, 16, 1) * 128.0).astype(np.float32).copy()
    sh["wp2m"] = np.clip(mpack(w_p2[1600:]) * 16.0, -15.5, 15.5)
    return sh


def _ensure_ntff_hook():
    """Dev-loop only: register the axon NTFF profile hook if the image's
    antenv lacks axon_hooks (concourse crashes on the import otherwise)."""
    try:
        from antenv.axon_hooks import get_axon_ntff_profile_hook  # noqa: F401
        return
    except ImportError:
        pass
    import types
    mod = types.ModuleType("antenv.axon_hooks")
    _hook = [None]
    mod.set_axon_ntff_profile_hook = lambda h: _hook.__setitem__(0, h)
    mod.get_axon_ntff_profile_hook = lambda: _hook[0]
    sys.modules["antenv.axon_hooks"] = mod
    try:
        import antenv
        antenv.axon_hooks = mod
    except ImportError:
        pass
    try:
        from trn_agent_boot.trn_boot import _ntff_profile_via_ctypes
        mod.set_axon_ntff_profile_hook(
            _ntff_profile_via_ctypes("/opt/axon/libaxon_pjrt.so"))
    except Exception as e:  # degrade: run untraced rather than crash
        print(f"ntff hook setup failed ({e}); running without trace")


def kernel(ec, eq, w_s, Wih0, Whh0, b0, Wih1, Whh1, b1, w_p1,
           Wih2, Whh2, b2, w_p2, b_p2, _t_len=T, _trace=False):
    if _trace:
        _ensure_ntff_hook()
    ec = np.asarray(ec, np.float32)
    eq = np.asarray(eq, np.float32)
    sh = _prep_shared(np.asarray(w_s), np.asarray(Wih0), np.asarray(Whh0), np.asarray(b0),
                      np.asarray(Wih1), np.asarray(Whh1), np.asarray(b1), np.asarray(w_p1),
                      np.asarray(Wih2), np.asarray(Whh2), np.asarray(b2), np.asarray(w_p2),
                      np.asarray(b_p2))
    if _t_len not in _CACHED:
        _CACHED[_t_len] = _build(_t_len)
    nc = _CACHED[_t_len]
    NT = _t_len // 128

    in_maps = []
    for b in range(B):
        im = dict(sh)
        ecb_ = ec[b, :_t_len]  # [T, 400]
        eqb_ = eq[b]
        im["ecT"] = ecb_.T.reshape(4, 100, _t_len).transpose(1, 0, 2).copy()
        im["ec"] = ecb_.reshape(NT, 128, D2).transpose(1, 0, 2).copy()
        im["eqT"] = eqb_.T.reshape(4, 100, J).transpose(1, 0, 2).copy()
        im["eq"] = eqb_.copy()
        in_maps.append(im)

    res = run_bass_kernel_spmd(nc, in_maps, list(range(B)), trace=_trace)
    kernel.last_exec_ns = res.exec_time_ns
    kernel.last_result = res
    p1 = np.zeros((B, _t_len), np.float32)
    p2 = np.zeros((B, _t_len), np.float32)
    for b in range(B):
        p1[b] = res.results[b]["p1out"][:, :NT].T.reshape(-1)
        p2[b] = res.results[b]["p2out"][:, :NT].T.reshape(-1)
    return (p1, p2)



# revision 30
# speedup vs baseline: 1.1329x; 1.1329x over previous
"""Trainium2 Bass kernel for nn_AttentionNet (BiDAF-style attention + 3 BiLSTM).

Data-parallel over batch B=8 across 8 NeuronCores; one batch element per core.
All tensors live feature-on-partition / T-on-free, so no transposes are needed
except tiny PE transposes inside the attention softmax.

LSTM recurrence: gates-on-partition layout. Gate vector (800) is permuted and
padded to 8 chunks of 128: [i0 i1 f0 f1 o0 o1 g0 g1] (k = unit index chunks
0:128 / 128:200+pad). Per tick and direction: 16 weight-stationary LDW+MM
pairs (k in {0,1} x m in 0..7), then sigmoid/tanh + cell update on 128
partitions. Padding rows self-clean (h_pad stays 0).
"""
import os
import sys
import numpy as np
import ml_dtypes

os.environ.setdefault("JAX_COMPILATION_CACHE_DIR", "/tmp/jax_neff_cache")
os.environ.setdefault("JAX_PERSISTENT_CACHE_MIN_COMPILE_TIME_SECS", "1")
os.environ.setdefault("JAX_PERSISTENT_CACHE_MIN_ENTRY_SIZE_BYTES", "0")
sys.path.insert(0, "/opt/trn_rl_repo")
from concourse import bacc, tile, mybir  # noqa: E402
from concourse.bass_utils import run_bass_kernel_spmd  # noqa: E402

dt = mybir.dt
AF = mybir.ActivationFunctionType
ALU = mybir.AluOpType

B, T, J = 8, 512, 64
H = 200
D2 = 400
NDIR = 2
BF = ml_dtypes.bfloat16

# gate permutation: old rows [i(200) f(200) g(200) o(200)] -> 8 chunks of 128
PERM_SRCS = [(0, 0, 128), (128, 128, 72), (200, 256, 128), (328, 384, 72),
             (600, 512, 128), (728, 640, 72), (400, 768, 128), (528, 896, 72)]
MBOUNDS = [(0, 128), (128, 200), (200, 328), (328, 400)]  # 400-dim k-chunk bounds


def perm_pad(vec800_last):
    out = np.zeros(vec800_last.shape[:-1] + (1024,), vec800_last.dtype)
    for so, do, n in PERM_SRCS:
        out[..., do:do + n] = vec800_last[..., so:so + n]
    return out


_CACHED = {}


def _build(t_len):
    nc = bacc.Bacc("TRN2", target_bir_lowering=False, debug=False, num_devices=8)
    NT = t_len // 128
    TPL = t_len + 4  # h column count (cols 0 and t_len+1 zero; +2 pad for 4B align)

    d = {}

    def dram(name, shape, dty=dt.float32, out=False):
        d[name] = nc.declare_dram_parameter(name, list(shape), dty, isOutput=out)
        return d[name]

    dram("ecT", [100, 4, t_len])          # ec transposed, feature chunks of 100
    dram("ec", [128, NT, D2])             # ec, T chunks of 128
    dram("eqT", [100, 4, J])
    dram("eq", [J, D2])
    dram("w1", [100, 4, 1])
    dram("w3", [100, 4, 1])
    dram("w2rep", [100, 4, 128])
    dram("ident", [128, 128])
    dram("ones", [128, 128])
    dram("wih0", [100, NDIR, 16, 1024], dt.bfloat16)
    dram("wih1", [128, NDIR, 4, 1024], dt.float8e3)
    dram("wih2", [128, NDIR, 4, 1024], dt.float8e3)
    for li in range(3):
        dram(f"whh{li}", [128, NDIR, 2, 1024], dt.float8e3)
        dram(f"b{li}", [128, NDIR, 8])
    dram("wp1g", [100, 16, 1])
    dram("wp1m", [128, 4, 1])
    dram("wp2g", [100, 16, 1])
    dram("wp2m", [128, 4, 1])
    dram("p1out", [128, NT], out=True)
    dram("p2out", [128, NT], out=True)

    with tile.TileContext(nc) as tc:
        with (
            tc.tile_pool(name="big", bufs=1) as big,
            tc.tile_pool(name="work", bufs=3) as work,
        ):
            # ---------- load inputs ----------
            ecT = big.tile([100, 4, t_len], dt.float32, name="ecT")
            ec = big.tile([128, NT, D2], dt.float32, name="ec")
            eqT = big.tile([100, 4, J], dt.float32, name="eqT")
            eq = big.tile([J, D2], dt.float32, name="eq")
            w1 = big.tile([100, 4, 1], dt.float32, name="w1")
            w3 = big.tile([100, 4, 1], dt.float32, name="w3")
            w2rep = big.tile([100, 4, 128], dt.float32, name="w2rep")
            ident = big.tile([128, 128], dt.float32, name="ident")
            ones = big.tile([128, 128], dt.float32, name="ones")
            for nm, tl in [("ecT", ecT), ("ec", ec), ("eqT", eqT), ("eq", eq),
                           ("w1", w1), ("w3", w3), ("w2rep", w2rep),
                           ("ident", ident), ("ones", ones)]:
                nc.sync.dma_start(out=tl[:], in_=d[nm].ap())

            ecTb = big.tile([100, 4, t_len], dt.bfloat16, name="ecTb")
            ecb = big.tile([128, NT, D2], dt.bfloat16, name="ecb")
            eqTb = big.tile([100, 4, J], dt.bfloat16, name="eqTb")
            eqb = big.tile([J, D2], dt.bfloat16, name="eqb")
            identb = big.tile([128, 128], dt.bfloat16, name="identb")
            w2repb = big.tile([100, 4, 128], dt.bfloat16, name="w2repb")
            ecw3Tb = big.tile([100, 4, t_len], dt.bfloat16, name="ecw3Tb")
            nc.any.tensor_copy(ecTb[:], ecT[:])
            nc.any.tensor_copy(ecb[:], ec[:])
            nc.any.tensor_copy(eqTb[:], eqT[:])
            nc.any.tensor_copy(eqb[:], eq[:])
            nc.any.tensor_copy(identb[:], ident[:])
            nc.any.tensor_copy(w2repb[:], w2rep[:])
            for fc in range(4):
                nc.vector.tensor_scalar_mul(ecw3Tb[:, fc], ecT[:, fc], w3[:, fc])

            # ---------- attention ----------
            Ptb = []
            mrow = []
            eb_all = big.tile([128, NT], dt.float32, name="eb_all")
            with tc.tile_pool(name="psA", bufs=1, space="PSUM") as ppa:
                for tch in range(NT):
                    sl = slice(tch * 128, (tch + 1) * 128)
                    ps_s = ppa.tile([128, J], dt.float32, name="ps_s", tag="ps_s")
                    for fc in range(4):
                        nc.tensor.matmul(ps_s[:], ecw3Tb[:, fc, sl], eqTb[:, fc],
                                         start=(fc == 0), stop=False)
                    for fc in range(4):
                        nc.tensor.matmul(ps_s[:], w2repb[:, fc], eqTb[:, fc],
                                         start=False, stop=(fc == 3))
                    ps_w1 = ppa.tile([128, 1], dt.float32, name="ps_w1", tag="ps_w1")
                    for fc in range(4):
                        nc.tensor.matmul(ps_w1[:], ecT[:, fc, sl], w1[:, fc],
                                         start=(fc == 0), stop=(fc == 3))
                    negr = work.tile([128, 1], dt.float32, name="negr", tag="negr")
                    nc.vector.tensor_reduce(negr[:], ps_s[:], mybir.AxisListType.X,
                                            ALU.max, negate=True)
                    e_t = work.tile([128, J], dt.float32, name="e_t", tag="e_t")
                    zsum = work.tile([128, 1], dt.float32, name="zsum", tag="zsum")
                    nc.scalar.activation(e_t[:], ps_s[:], AF.Exp, bias=negr[:],
                                         accum_out=zsum[:])
                    invz = work.tile([128, 1], dt.float32, name="invz", tag="invz")
                    nc.vector.reciprocal(invz[:], zsum[:])
                    pnorm = work.tile([128, J], dt.bfloat16, name="pnorm", tag="pnorm")
                    nc.vector.tensor_scalar_mul(pnorm[:], e_t[:], invz[:])
                    mr = work.tile([128, 1], dt.float32, name="mr", tag=f"mr{tch}")
                    nc.vector.scalar_tensor_tensor(mr[:], negr[:], -1.0, ps_w1[:],
                                                   ALU.mult, ALU.add)
                    mrow.append(mr)
                    ps_pt = ppa.tile([J, 128], dt.bfloat16, name="ps_pt", tag="ps_pt")
                    nc.tensor.transpose(ps_pt[:], pnorm[:], identb[:])
                    ptb = big.tile([J, 128], dt.bfloat16, name=f"ptb{tch}")
                    nc.any.tensor_copy(ptb[:], ps_pt[:])
                    Ptb.append(ptb)

                for tch in range(NT):
                    nc.scalar.activation(eb_all[:, tch:tch + 1], mrow[tch][:], AF.Exp)
                ps_zb = ppa.tile([1, NT], dt.float32, name="ps_zb", tag="ps_zb")
                nc.tensor.matmul(ps_zb[:], ones[:, 0:1], eb_all[:], start=True, stop=True)
                zb = work.tile([1, 1], dt.float32, name="zb")
                nc.vector.tensor_reduce(zb[:], ps_zb[:], mybir.AxisListType.X, ALU.add)
                invzb = work.tile([1, 1], dt.float32, name="invzb")
                nc.vector.reciprocal(invzb[:], zb[:])
                ps_izb = ppa.tile([128, 1], dt.float32, name="ps_izb", tag="ps_izb")
                nc.tensor.matmul(ps_izb[:], ones[0:1, :], invzb[:], start=True, stop=True)
                izb = work.tile([128, 1], dt.float32, name="izb")
                nc.any.tensor_copy(izb[:], ps_izb[:])
                ebn = big.tile([128, NT], dt.float32, name="ebn")
                nc.vector.tensor_scalar_mul(ebn[:], eb_all[:], izb[:])

                q2cT = big.tile([100, 4, 1], dt.float32, name="q2cT")
                for fc in range(4):
                    ps_q = ppa.tile([100, 1], dt.float32, name="ps_q", tag="ps_q")
                    for tch in range(NT):
                        nc.tensor.matmul(ps_q[:], ec[:, tch, fc * 100:(fc + 1) * 100],
                                         ebn[:, tch:tch + 1],
                                         start=(tch == 0), stop=(tch == NT - 1))
                    nc.any.tensor_copy(q2cT[:, fc], ps_q[:])

                Gtiles = [ecTb[:, fc] for fc in range(4)]
                g_c2q, g_pc, g_pq = [], [], []
                for fc in range(4):
                    ps_c = ppa.tile([100, t_len], dt.float32, name="ps_c", tag="ps_c")
                    for tch in range(NT):
                        nc.tensor.matmul(ps_c[:, tch * 128:(tch + 1) * 128],
                                         eqb[:, fc * 100:(fc + 1) * 100], Ptb[tch][:],
                                         start=True, stop=True)
                    c2q_f = big.tile([100, t_len], dt.bfloat16, name=f"c2q{fc}")
                    nc.any.tensor_copy(c2q_f[:], ps_c[:])
                    g2 = big.tile([100, t_len], dt.bfloat16, name=f"g2_{fc}")
                    nc.vector.scalar_tensor_tensor(g2[:], ps_c[:], 1.0, ecT[:, fc],
                                                   ALU.mult, ALU.mult)
                    g3 = big.tile([100, t_len], dt.bfloat16, name=f"g3_{fc}")
                    nc.vector.tensor_scalar_mul(g3[:], ecT[:, fc], q2cT[:, fc])
                    g_c2q.append(c2q_f)
                    g_pc.append(g2)
                    g_pq.append(g3)
                Gtiles += [x[:] for x in g_c2q] + [x[:] for x in g_pc] + [x[:] for x in g_pq]

            # ---------- LSTM stack ----------
            def lstm_layer(li, in_tiles, in_kdims, nk_wih, kpart):
                wih_dt = dt.bfloat16 if li == 0 else dt.float8e3
                whh = [big.tile([128, 2, 1024], dt.float8e3, name=f"whh{li}_{dd}", tag=f"whhS_{dd}")
                       for dd in range(NDIR)]
                bvec = [big.tile([128, 8], dt.float32, name=f"b{li}_{dd}", tag=f"bS_{dd}")
                        for dd in range(NDIR)]
                wih = [big.tile([kpart, nk_wih, 1024], wih_dt, name=f"wih{li}_{dd}", tag=f"wihS_{dd}")
                       for dd in range(NDIR)]
                for dd in range(NDIR):
                    nc.sync.dma_start(out=whh[dd][:], in_=d[f"whh{li}"].ap()[:, dd])
                    nc.sync.dma_start(out=bvec[dd][:], in_=d[f"b{li}"].ap()[:, dd])
                    nc.sync.dma_start(out=wih[dd][:], in_=d[f"wih{li}"].ap()[:, dd])

                pre = [big.tile([128, t_len, 8], dt.bfloat16, name=f"pre{li}_{dd}", tag=f"preS_{dd}")
                       for dd in range(NDIR)]
                with tc.tile_pool(name=f"psP{li}", bufs=2, space="PSUM") as ppp:
                    for dd in range(NDIR):
                        for m in range(8):
                            ps_p = ppp.tile([128, t_len], dt.float32, name="ps_p", tag="ps_p")
                            for ki, (ap_k, kdim) in enumerate(zip(in_tiles, in_kdims)):
                                nc.tensor.matmul(
                                    ps_p[:],
                                    wih[dd][0:kdim, ki, m * 128:(m + 1) * 128],
                                    ap_k[0:kdim, :],
                                    start=(ki == 0), stop=(ki == len(in_tiles) - 1))
                            nc.vector.tensor_scalar_add(pre[dd][:, :, m], ps_p[:],
                                                        bvec[dd][:, m:m + 1])

                # x8-scaled fp8 h, flat [units, time*4] with col t*4+half: the
                # only h storage. Cols t*4+2/3 are junk lanes so the per-tick
                # store is a 4-byte-aligned [128, 4] write; readers use
                # stride-4 slices. Later layers/heads read stride-4 time slices.
                h8 = [big.tile([128, TPL * 4], dt.float8e3, name=f"h8{li}_{dd}", tag=f"h8S{li}_{dd}")
                      for dd in range(NDIR)]
                for dd in range(NDIR):
                    nc.any.memzero(h8[dd][:])
                c_prev = []
                for dd in range(NDIR):
                    c0 = work.tile([128, 4], dt.float32, name=f"c0_{dd}", tag=f"c{li}_{dd}x1")
                    nc.any.memzero(c0[:])
                    c_prev.append(c0)
                # s8 gets 2 zero pad columns (8:10) so 4-wide slices starting
                # at col 6 stay in bounds; zero once per layer per dir.
                s8 = [work.tile([128, 10], dt.float32, name=f"s8_{dd}", tag=f"s{li}_{dd}")
                      for dd in range(NDIR)]
                for dd in range(NDIR):
                    nc.any.memzero(s8[dd][:])

                with tc.tile_pool(name=f"psR{li}", bufs=2, space="PSUM") as ppr:
                    for tt in range(t_len):
                        for dd in range(NDIR):
                            if dd == 0:
                                t_idx = tt
                                rd_col, wr_col = t_idx, t_idx + 1
                            else:
                                t_idx = t_len - 1 - tt
                                rd_col, wr_col = t_idx + 2, t_idx + 1
                            ps = ppr.tile([128, 8], dt.float32, name=f"ps{dd}", tag=f"ps{dd}")
                            rhs0 = h8[dd][:, rd_col * 4:rd_col * 4 + 1]
                            rhs1 = h8[dd][:, rd_col * 4 + 1:rd_col * 4 + 2]
                            if tt < 2:
                                # prime this PSUM buffer's has_written bits
                                # with a real accumulation group
                                nc.tensor.matmul(ps[:], identb[:], pre[dd][:, t_idx, :],
                                                 start=True, stop=False)
                                for m in range(8):
                                    nc.tensor.matmul(ps[:, m:m + 1],
                                                     whh[dd][:, 0, m * 128:(m + 1) * 128],
                                                     rhs0, start=False, stop=False)
                                    nc.tensor.matmul(ps[:, m:m + 1],
                                                     whh[dd][:, 1, m * 128:(m + 1) * 128],
                                                     rhs1, start=False, stop=(m == 7))
                            else:
                                # pre lands in PSUM via DVE; matmuls accumulate
                                # onto it (bank bits stay "written" since the
                                # priming group, so start=False adds)
                                nc.vector.tensor_copy(ps[:], pre[dd][:, t_idx, :])
                                for m in range(8):
                                    nc.tensor.matmul(ps[:, m:m + 1],
                                                     whh[dd][:, 0, m * 128:(m + 1) * 128],
                                                     rhs0, start=False, stop=False,
                                                     skip_group_check=True)
                                    nc.tensor.matmul(ps[:, m:m + 1],
                                                     whh[dd][:, 1, m * 128:(m + 1) * 128],
                                                     rhs1, start=False, stop=False,
                                                     skip_group_check=True)
                            # s8[0:8] = sigmoid of the 8 gate chunks (x256 scale,
                            # g chunks additionally x2); cols 8:10 stay zero pad.
                            # Chain ops below run 4-wide (cols 2:4 of each are
                            # junk lanes) so the h8 store is one aligned 4B write.
                            nc.scalar.activation(s8[dd][:, 0:8], ps[:], AF.Sigmoid,
                                                 scale=0.00390625)
                            sd = s8[dd]
                            t1 = work.tile([128, 4], dt.float32, name=f"t1{dd}", tag=f"t1{li}_{dd}")
                            u = work.tile([128, 4], dt.float32, name=f"u{dd}", tag=f"u{li}_{dd}")
                            v = work.tile([128, 4], dt.float32, name=f"v{dd}", tag=f"v{li}_{dd}")
                            c_new = work.tile([128, 4], dt.float32, name=f"cn{dd}",
                                              tag=f"c{li}_{dd}x{tt % 2}")
                            # t1 = 2*sg*si ; u = t1 - si   (tanh(g)=2*sigmoid(2g)-1)
                            nc.vector.scalar_tensor_tensor(t1[:], sd[:, 6:10], 2.0, sd[:, 0:4],
                                                           ALU.mult, ALU.mult)
                            nc.vector.scalar_tensor_tensor(u[:], t1[:], 1.0, sd[:, 0:4],
                                                           ALU.mult, ALU.subtract)
                            nc.vector.scalar_tensor_tensor(v[:], c_prev[dd][:], 0.0, sd[:, 2:6],
                                                           ALU.add, ALU.mult)
                            nc.vector.scalar_tensor_tensor(c_new[:], u[:], 0.0, v[:],
                                                           ALU.add, ALU.add)
                            # h8 = 8 * tanh(c) * so
                            th = work.tile([128, 4], dt.float32, name=f"th{dd}", tag=f"th{li}_{dd}")
                            nc.scalar.activation(th[:], c_new[:], AF.Tanh)
                            nc.vector.scalar_tensor_tensor(h8[dd][:, wr_col * 4:wr_col * 4 + 4],
                                                           th[:], 8.0, sd[:, 4:8],
                                                           ALU.mult, ALU.mult)
                            c_prev[dd] = c_new
                return h8

            def h8_tiles(h8):
                # [fwd_lo, fwd_hi, bwd_lo, bwd_hi] over the valid time range
                a, b_ = 4, 4 + 4 * t_len
                return [h8[0][:, a:b_:4], h8[0][0:72, a + 1:b_:4],
                        h8[1][:, a:b_:4], h8[1][0:72, a + 1:b_:4]]

            h0 = lstm_layer(0, Gtiles, [100] * 16, 16, 100)
            mk = [128, 72, 128, 72]
            m0_tiles = h8_tiles(h0)
            h1 = lstm_layer(1, m0_tiles, mk, 4, 128)
            m1_tiles = h8_tiles(h1)
            h2 = lstm_layer(2, m1_tiles, mk, 4, 128)
            m2_tiles = h8_tiles(h2)

            # ---------- heads ----------
            # logits carry a x128 scale: G-part weights are x128 (bf16),
            # M-part weights are x16 (fp8e3) against the x8-scaled h8; the
            # exp undoes it with scale=1/128.
            wpb = {}
            for nm, shp in [("wp1g", [100, 16, 1]), ("wp1m", [128, 4, 1]),
                            ("wp2g", [100, 16, 1]), ("wp2m", [128, 4, 1])]:
                tl = work.tile(shp, dt.float32, name=nm, tag=nm)
                nc.sync.dma_start(out=tl[:], in_=d[nm].ap())
                wdt = dt.float8e3 if nm.endswith("m") else dt.bfloat16
                tb = work.tile(shp, wdt, name=nm + "b", tag=nm + "b")
                nc.any.tensor_copy(tb[:], tl[:])
                wpb[nm] = tb

            with tc.tile_pool(name="psH", bufs=2, space="PSUM") as pph:
                def head(gname, mname, m_tiles, out_name):
                    e_all = work.tile([128, NT], dt.float32, name=f"e_{gname}")
                    for tch2 in range(NT):
                        sl = slice(tch2 * 128, (tch2 + 1) * 128)
                        ps_l = pph.tile([128, 1], dt.float32, name="ps_l", tag="ps_l")
                        for gi in range(16):
                            nc.tensor.matmul(ps_l[:], Gtiles[gi][:, sl], wpb[gname][:, gi],
                                             start=(gi == 0), stop=False)
                        for ki in range(4):
                            nc.tensor.matmul(ps_l[:], m_tiles[ki][0:mk[ki], sl],
                                             wpb[mname][0:mk[ki], ki],
                                             start=False, stop=(ki == 3))
                        nc.scalar.activation(e_all[:, tch2:tch2 + 1], ps_l[:], AF.Exp,
                                             scale=0.0078125)
                    ps_z = pph.tile([1, NT], dt.float32, name="ps_z", tag="ps_z")
                    nc.tensor.matmul(ps_z[:], ones[:, 0:1], e_all[:], start=True, stop=True)
                    z = work.tile([1, 1], dt.float32, name=f"z_{gname}")
                    nc.vector.tensor_reduce(z[:], ps_z[:], mybir.AxisListType.X, ALU.add)
                    iz = work.tile([1, 1], dt.float32, name=f"iz_{gname}")
                    nc.vector.reciprocal(iz[:], z[:])
                    ps_i = pph.tile([128, 1], dt.float32, name="ps_i", tag="ps_i")
                    nc.tensor.matmul(ps_i[:], ones[0:1, :], iz[:], start=True, stop=True)
                    izr = work.tile([128, 1], dt.float32, name=f"izr_{gname}")
                    nc.any.tensor_copy(izr[:], ps_i[:])
                    pout = work.tile([128, NT], dt.float32, name=f"pout_{gname}")
                    nc.vector.tensor_scalar_mul(pout[:], e_all[:], izr[:])
                    nc.sync.dma_start(out=d[out_name].ap(), in_=pout[:])

                head("wp1g", "wp1m", m1_tiles, "p1out")
                head("wp2g", "wp2m", m2_tiles, "p2out")

    nc.compile()
    return nc


def _prep_shared(w_s, Wih0, Whh0, b0, Wih1, Whh1, b1, w_p1, Wih2, Whh2, b2, w_p2, b_p2):
    sh = {}
    w1v, w2v, w3v = w_s[:D2], w_s[D2:2 * D2], w_s[2 * D2:]
    sh["w1"] = w1v.reshape(4, 100).T.reshape(100, 4, 1).astype(np.float32).copy()
    sh["w3"] = w3v.reshape(4, 100).T.reshape(100, 4, 1).astype(np.float32).copy()
    sh["w2rep"] = np.repeat(w2v.reshape(4, 100).T.reshape(100, 4, 1), 128, axis=2).astype(np.float32)
    sh["ident"] = np.eye(128, dtype=np.float32)
    sh["ones"] = np.ones((128, 128), np.float32)

    # whh carries x32 (fits fp8e3 normal range), h8 carries x8, so the full
    # recurrent product is x256; wih/b are x256 so preactivations match, and
    # the scan's sigmoid uses scale=1/256 to undo it.
    QW, QP = 32.0, 256.0

    def whh_pack(Whh):
        WP = perm_pad(np.swapaxes(Whh, 1, 2))  # [2, 200, 1024]
        WP[..., 768:1024] *= 2.0
        out = np.zeros((128, NDIR, 2, 1024), np.float32)
        out[:, :, 0] = WP[:, 0:128].transpose(1, 0, 2)
        out[0:72, :, 1] = WP[:, 128:200].transpose(1, 0, 2)
        return np.clip(out * QW, -15.5, 15.5).astype(ml_dtypes.float8_e3m4)

    def bias_pack(b):
        bP = perm_pad(b)  # [2, 1024]
        bP[..., 768:1024] *= 2.0
        return (bP.reshape(NDIR, 8, 128).transpose(2, 0, 1) * QP).copy().astype(np.float32)

    def wih_pack(Wih, nk, kdim, fp8=False):
        WP = perm_pad(np.swapaxes(Wih, 1, 2))  # [2, IN, 1024]
        WP[..., 768:1024] *= 2.0
        out = np.zeros((kdim, NDIR, nk, 1024), np.float32)
        if kdim == 100:
            for k in range(nk):
                out[:, :, k] = WP[:, k * 100:(k + 1) * 100].transpose(1, 0, 2)
        else:
            for k, (a, b_) in enumerate(MBOUNDS):
                out[0:b_ - a, :, k] = WP[:, a:b_].transpose(1, 0, 2)
        if fp8:  # consumed against x8-scaled h8 -> x32 weight keeps x256 total
            return np.clip(out * QW, -15.5, 15.5).astype(ml_dtypes.float8_e3m4)
        return (out * QP).astype(BF)

    sh["wih0"] = wih_pack(Wih0, 16, 100)
    sh["wih1"] = wih_pack(Wih1, 4, 128, fp8=True)
    sh["wih2"] = wih_pack(Wih2, 4, 128, fp8=True)
    sh["whh0"] = whh_pack(Whh0)
    sh["whh1"] = whh_pack(Whh1)
    sh["whh2"] = whh_pack(Whh2)
    sh["b0"] = bias_pack(b0)
    sh["b1"] = bias_pack(b1)
    sh["b2"] = bias_pack(b2)

    def mpack(wm):
        out = np.zeros((128, 4, 1), np.float32)
        for k, (a, b_) in enumerate(MBOUNDS):
            out[0:b_ - a, k, 0] = wm[a:b_]
        return out

    # head logit scale: G-part x128, M-part x16 (reads x8-scaled h8)
    sh["wp1g"] = (w_p1[:1600].reshape(16, 100).T.reshape(100, 16, 1) * 128.0).astype(np.float32).copy()
    sh["wp1m"] = np.clip(mpack(w_p1[1600:]) * 16.0, -15.5, 15.5)
    sh["wp2g"] = (w_p2[:1600].reshape(16, 100).T.reshape(100, 16, 1) * 128.0).astype(np.float32).copy()
    sh["wp2m"] = np.clip(mpack(w_p2[1600:]) * 16.0, -15.5, 15.5)
    return sh


def _ensure_ntff_hook():
    """Dev-loop only: register the axon NTFF profile hook if the image's
    antenv lacks axon_hooks (concourse crashes on the import otherwise)."""
    try:
        from antenv.axon_hooks import get_axon_ntff_profile_hook  # noqa: F401
        return
    except ImportError:
        pass
    import types
    mod = types.ModuleType("antenv.axon_hooks")
    _hook = [None]
    mod.set_axon_ntff_profile_hook = lambda h: _hook.__setitem__(0, h)
    mod.get_axon_ntff_profile_hook = lambda: _hook[0]
    sys.modules["antenv.axon_hooks"] = mod
    try:
        import antenv
        antenv.axon_hooks = mod
    except ImportError:
        pass
    try:
        from trn_agent_boot.trn_boot import _ntff_profile_via_ctypes
        mod.set_axon_ntff_profile_hook(
            _ntff_profile_via_ctypes("/opt/axon/libaxon_pjrt.so"))
    except Exception as e:  # degrade: run untraced rather than crash
        print(f"ntff hook setup failed ({e}); running without trace")


def kernel(ec, eq, w_s, Wih0, Whh0, b0, Wih1, Whh1, b1, w_p1,
           Wih2, Whh2, b2, w_p2, b_p2, _t_len=T, _trace=False):
    if _trace:
        _ensure_ntff_hook()
    ec = np.asarray(ec, np.float32)
    eq = np.asarray(eq, np.float32)
    sh = _prep_shared(np.asarray(w_s), np.asarray(Wih0), np.asarray(Whh0), np.asarray(b0),
                      np.asarray(Wih1), np.asarray(Whh1), np.asarray(b1), np.asarray(w_p1),
                      np.asarray(Wih2), np.asarray(Whh2), np.asarray(b2), np.asarray(w_p2),
                      np.asarray(b_p2))
    if _t_len not in _CACHED:
        _CACHED[_t_len] = _build(_t_len)
    nc = _CACHED[_t_len]
    NT = _t_len // 128

    in_maps = []
    for b in range(B):
        im = dict(sh)
        ecb_ = ec[b, :_t_len]  # [T, 400]
        eqb_ = eq[b]
        im["ecT"] = ecb_.T.reshape(4, 100, _t_len).transpose(1, 0, 2).copy()
        im["ec"] = ecb_.reshape(NT, 128, D2).transpose(1, 0, 2).copy()
        im["eqT"] = eqb_.T.reshape(4, 100, J).transpose(1, 0, 2).copy()
        im["eq"] = eqb_.copy()
        in_maps.append(im)

    res = run_bass_kernel_spmd(nc, in_maps, list(range(B)), trace=_trace)
    kernel.last_exec_ns = res.exec_time_ns
    kernel.last_result = res
    p1 = np.zeros((B, _t_len), np.float32)
    p2 = np.zeros((B, _t_len), np.float32)
    for b in range(B):
        p1[b] = res.results[b]["p1out"][:, :NT].T.reshape(-1)
        p2[b] = res.results[b]["p2out"][:, :NT].T.reshape(-1)
    return (p1, p2)



# revision 35
# speedup vs baseline: 1.1440x; 1.0098x over previous
"""Trainium2 Bass kernel for nn_AttentionNet (BiDAF-style attention + 3 BiLSTM).

Data-parallel over batch B=8 across 8 NeuronCores; one batch element per core.
All tensors live feature-on-partition / T-on-free, so no transposes are needed
except tiny PE transposes inside the attention softmax.

LSTM recurrence: gates-on-partition layout. Gate vector (800) is permuted and
padded to 8 chunks of 128: [i0 i1 f0 f1 o0 o1 g0 g1] (k = unit index chunks
0:128 / 128:200+pad). Per tick and direction: 16 weight-stationary LDW+MM
pairs (k in {0,1} x m in 0..7), then sigmoid/tanh + cell update on 128
partitions. Padding rows self-clean (h_pad stays 0).
"""
import os
import sys
import numpy as np
import ml_dtypes

os.environ.setdefault("JAX_COMPILATION_CACHE_DIR", "/tmp/jax_neff_cache")
os.environ.setdefault("JAX_PERSISTENT_CACHE_MIN_COMPILE_TIME_SECS", "1")
os.environ.setdefault("JAX_PERSISTENT_CACHE_MIN_ENTRY_SIZE_BYTES", "0")
sys.path.insert(0, "/opt/trn_rl_repo")
from concourse import bacc, tile, mybir  # noqa: E402
from concourse.bass_utils import run_bass_kernel_spmd  # noqa: E402

dt = mybir.dt
AF = mybir.ActivationFunctionType
ALU = mybir.AluOpType

B, T, J = 8, 512, 64
H = 200
D2 = 400
NDIR = 2
BF = ml_dtypes.bfloat16

# gate permutation: old rows [i(200) f(200) g(200) o(200)] -> 8 chunks of 128
PERM_SRCS = [(0, 0, 128), (128, 128, 72), (200, 256, 128), (328, 384, 72),
             (600, 512, 128), (728, 640, 72), (400, 768, 128), (528, 896, 72)]
MBOUNDS = [(0, 128), (128, 200), (200, 328), (328, 400)]  # 400-dim k-chunk bounds


def perm_pad(vec800_last):
    out = np.zeros(vec800_last.shape[:-1] + (1024,), vec800_last.dtype)
    for so, do, n in PERM_SRCS:
        out[..., do:do + n] = vec800_last[..., so:so + n]
    return out


_CACHED = {}


def _build(t_len):
    nc = bacc.Bacc("TRN2", target_bir_lowering=False, debug=False, num_devices=8)
    NT = t_len // 128
    TPL = t_len + 4  # h column count (cols 0 and t_len+1 zero; +2 pad for 4B align)

    d = {}

    def dram(name, shape, dty=dt.float32, out=False):
        d[name] = nc.declare_dram_parameter(name, list(shape), dty, isOutput=out)
        return d[name]

    dram("ecT", [100, 4, t_len])          # ec transposed, feature chunks of 100
    dram("ec", [128, NT, D2])             # ec, T chunks of 128
    dram("eqT", [100, 4, J])
    dram("eq", [J, D2])
    dram("w1", [100, 4, 1])
    dram("w3", [100, 4, 1])
    dram("w2rep", [100, 4, 128])
    dram("ident", [128, 128])
    dram("ones", [128, 128])
    dram("wih0", [100, NDIR, 16, 1024], dt.bfloat16)
    dram("wih1", [128, NDIR, 4, 1024], dt.float8e3)
    dram("wih2", [128, NDIR, 4, 1024], dt.float8e3)
    for li in range(3):
        dram(f"whh{li}", [128, NDIR, 2, 1024], dt.float8e3)
        dram(f"b{li}", [128, NDIR, 8])
    dram("wp1g", [100, 16, 1])
    dram("wp1m", [128, 4, 1])
    dram("wp2g", [100, 16, 1])
    dram("wp2m", [128, 4, 1])
    dram("p1out", [128, NT], out=True)
    dram("p2out", [128, NT], out=True)

    with tile.TileContext(nc) as tc:
        with (
            tc.tile_pool(name="big", bufs=1) as big,
            tc.tile_pool(name="work", bufs=3) as work,
        ):
            # ---------- load inputs ----------
            ecT = big.tile([100, 4, t_len], dt.float32, name="ecT")
            ec = big.tile([128, NT, D2], dt.float32, name="ec")
            eqT = big.tile([100, 4, J], dt.float32, name="eqT")
            eq = big.tile([J, D2], dt.float32, name="eq")
            w1 = big.tile([100, 4, 1], dt.float32, name="w1")
            w3 = big.tile([100, 4, 1], dt.float32, name="w3")
            w2rep = big.tile([100, 4, 128], dt.float32, name="w2rep")
            ident = big.tile([128, 128], dt.float32, name="ident")
            ones = big.tile([128, 128], dt.float32, name="ones")
            for nm, tl in [("ecT", ecT), ("ec", ec), ("eqT", eqT), ("eq", eq),
                           ("w1", w1), ("w3", w3), ("w2rep", w2rep),
                           ("ident", ident), ("ones", ones)]:
                nc.sync.dma_start(out=tl[:], in_=d[nm].ap())

            ecTb = big.tile([100, 4, t_len], dt.bfloat16, name="ecTb")
            ecb = big.tile([128, NT, D2], dt.bfloat16, name="ecb")
            eqTb = big.tile([100, 4, J], dt.bfloat16, name="eqTb")
            eqb = big.tile([J, D2], dt.bfloat16, name="eqb")
            identb = big.tile([128, 128], dt.bfloat16, name="identb")
            w2repb = big.tile([100, 4, 128], dt.bfloat16, name="w2repb")
            ecw3Tb = big.tile([100, 4, t_len], dt.bfloat16, name="ecw3Tb")
            nc.any.tensor_copy(ecTb[:], ecT[:])
            nc.any.tensor_copy(ecb[:], ec[:])
            nc.any.tensor_copy(eqTb[:], eqT[:])
            nc.any.tensor_copy(eqb[:], eq[:])
            nc.any.tensor_copy(identb[:], ident[:])
            nc.any.tensor_copy(w2repb[:], w2rep[:])
            for fc in range(4):
                nc.vector.tensor_scalar_mul(ecw3Tb[:, fc], ecT[:, fc], w3[:, fc])

            # ---------- attention ----------
            Ptb = []
            mrow = []
            eb_all = big.tile([128, NT], dt.float32, name="eb_all")
            with tc.tile_pool(name="psA", bufs=1, space="PSUM") as ppa:
                for tch in range(NT):
                    sl = slice(tch * 128, (tch + 1) * 128)
                    ps_s = ppa.tile([128, J], dt.float32, name="ps_s", tag="ps_s")
                    for fc in range(4):
                        nc.tensor.matmul(ps_s[:], ecw3Tb[:, fc, sl], eqTb[:, fc],
                                         start=(fc == 0), stop=False)
                    for fc in range(4):
                        nc.tensor.matmul(ps_s[:], w2repb[:, fc], eqTb[:, fc],
                                         start=False, stop=(fc == 3))
                    ps_w1 = ppa.tile([128, 1], dt.float32, name="ps_w1", tag="ps_w1")
                    for fc in range(4):
                        nc.tensor.matmul(ps_w1[:], ecT[:, fc, sl], w1[:, fc],
                                         start=(fc == 0), stop=(fc == 3))
                    negr = work.tile([128, 1], dt.float32, name="negr", tag="negr")
                    nc.vector.tensor_reduce(negr[:], ps_s[:], mybir.AxisListType.X,
                                            ALU.max, negate=True)
                    e_t = work.tile([128, J], dt.float32, name="e_t", tag="e_t")
                    zsum = work.tile([128, 1], dt.float32, name="zsum", tag="zsum")
                    nc.scalar.activation(e_t[:], ps_s[:], AF.Exp, bias=negr[:],
                                         accum_out=zsum[:])
                    invz = work.tile([128, 1], dt.float32, name="invz", tag="invz")
                    nc.vector.reciprocal(invz[:], zsum[:])
                    pnorm = work.tile([128, J], dt.bfloat16, name="pnorm", tag="pnorm")
                    nc.vector.tensor_scalar_mul(pnorm[:], e_t[:], invz[:])
                    mr = work.tile([128, 1], dt.float32, name="mr", tag=f"mr{tch}")
                    nc.vector.scalar_tensor_tensor(mr[:], negr[:], -1.0, ps_w1[:],
                                                   ALU.mult, ALU.add)
                    mrow.append(mr)
                    ps_pt = ppa.tile([J, 128], dt.bfloat16, name="ps_pt", tag="ps_pt")
                    nc.tensor.transpose(ps_pt[:], pnorm[:], identb[:])
                    ptb = big.tile([J, 128], dt.bfloat16, name=f"ptb{tch}")
                    nc.any.tensor_copy(ptb[:], ps_pt[:])
                    Ptb.append(ptb)

                for tch in range(NT):
                    nc.scalar.activation(eb_all[:, tch:tch + 1], mrow[tch][:], AF.Exp)
                ps_zb = ppa.tile([1, NT], dt.float32, name="ps_zb", tag="ps_zb")
                nc.tensor.matmul(ps_zb[:], ones[:, 0:1], eb_all[:], start=True, stop=True)
                zb = work.tile([1, 1], dt.float32, name="zb")
                nc.vector.tensor_reduce(zb[:], ps_zb[:], mybir.AxisListType.X, ALU.add)
                invzb = work.tile([1, 1], dt.float32, name="invzb")
                nc.vector.reciprocal(invzb[:], zb[:])
                ps_izb = ppa.tile([128, 1], dt.float32, name="ps_izb", tag="ps_izb")
                nc.tensor.matmul(ps_izb[:], ones[0:1, :], invzb[:], start=True, stop=True)
                izb = work.tile([128, 1], dt.float32, name="izb")
                nc.any.tensor_copy(izb[:], ps_izb[:])
                ebn = big.tile([128, NT], dt.float32, name="ebn")
                nc.vector.tensor_scalar_mul(ebn[:], eb_all[:], izb[:])

                q2cT = big.tile([100, 4, 1], dt.float32, name="q2cT")
                for fc in range(4):
                    ps_q = ppa.tile([100, 1], dt.float32, name="ps_q", tag="ps_q")
                    for tch in range(NT):
                        nc.tensor.matmul(ps_q[:], ec[:, tch, fc * 100:(fc + 1) * 100],
                                         ebn[:, tch:tch + 1],
                                         start=(tch == 0), stop=(tch == NT - 1))
                    nc.any.tensor_copy(q2cT[:, fc], ps_q[:])

                Gtiles = [ecTb[:, fc] for fc in range(4)]
                g_c2q, g_pc, g_pq = [], [], []
                for fc in range(4):
                    ps_c = ppa.tile([100, t_len], dt.float32, name="ps_c", tag="ps_c")
                    for tch in range(NT):
                        nc.tensor.matmul(ps_c[:, tch * 128:(tch + 1) * 128],
                                         eqb[:, fc * 100:(fc + 1) * 100], Ptb[tch][:],
                                         start=True, stop=True)
                    c2q_f = big.tile([100, t_len], dt.bfloat16, name=f"c2q{fc}")
                    nc.any.tensor_copy(c2q_f[:], ps_c[:])
                    g2 = big.tile([100, t_len], dt.bfloat16, name=f"g2_{fc}")
                    nc.vector.scalar_tensor_tensor(g2[:], ps_c[:], 1.0, ecT[:, fc],
                                                   ALU.mult, ALU.mult)
                    g3 = big.tile([100, t_len], dt.bfloat16, name=f"g3_{fc}")
                    nc.vector.tensor_scalar_mul(g3[:], ecT[:, fc], q2cT[:, fc])
                    g_c2q.append(c2q_f)
                    g_pc.append(g2)
                    g_pq.append(g3)
                Gtiles += [x[:] for x in g_c2q] + [x[:] for x in g_pc] + [x[:] for x in g_pq]

            # ---------- LSTM stack ----------
            def lstm_layer(li, in_tiles, in_kdims, nk_wih, kpart):
                wih_dt = dt.bfloat16 if li == 0 else dt.float8e3
                whh = [big.tile([128, 2, 1024], dt.float8e3, name=f"whh{li}_{dd}", tag=f"whhS_{dd}")
                       for dd in range(NDIR)]
                bvec = [big.tile([128, 8], dt.float32, name=f"b{li}_{dd}", tag=f"bS_{dd}")
                        for dd in range(NDIR)]
                wih = [big.tile([kpart, nk_wih, 1024], wih_dt, name=f"wih{li}_{dd}", tag=f"wihS_{dd}")
                       for dd in range(NDIR)]
                for dd in range(NDIR):
                    nc.sync.dma_start(out=whh[dd][:], in_=d[f"whh{li}"].ap()[:, dd])
                    nc.sync.dma_start(out=bvec[dd][:], in_=d[f"b{li}"].ap()[:, dd])
                    nc.sync.dma_start(out=wih[dd][:], in_=d[f"wih{li}"].ap()[:, dd])

                # m-major: the 8 per-m bias-adds write contiguous [128, T]
                # rows; the per-tick reads are 8-element strided (init-bound)
                pre = [big.tile([128, 8, t_len], dt.bfloat16, name=f"pre{li}_{dd}", tag=f"preS_{dd}")
                       for dd in range(NDIR)]
                with tc.tile_pool(name=f"psP{li}", bufs=2, space="PSUM") as ppp:
                    for dd in range(NDIR):
                        for m in range(8):
                            ps_p = ppp.tile([128, t_len], dt.float32, name="ps_p", tag="ps_p")
                            for ki, (ap_k, kdim) in enumerate(zip(in_tiles, in_kdims)):
                                nc.tensor.matmul(
                                    ps_p[:],
                                    wih[dd][0:kdim, ki, m * 128:(m + 1) * 128],
                                    ap_k[0:kdim, :],
                                    start=(ki == 0), stop=(ki == len(in_tiles) - 1))
                            nc.vector.tensor_scalar_add(pre[dd][:, m, :], ps_p[:],
                                                        bvec[dd][:, m:m + 1])

                # x8-scaled fp8 h, flat [units, time*4] with col t*4+half: the
                # only h storage. Cols t*4+2/3 are junk lanes so the per-tick
                # store is a 4-byte-aligned [128, 4] write; readers use
                # stride-4 slices. Later layers/heads read stride-4 time slices.
                h8 = [big.tile([128, TPL * 4], dt.float8e3, name=f"h8{li}_{dd}", tag=f"h8S{li}_{dd}")
                      for dd in range(NDIR)]
                for dd in range(NDIR):
                    nc.any.memzero(h8[dd][:])
                c_prev = []
                for dd in range(NDIR):
                    c0 = work.tile([128, 4], dt.float32, name=f"c0_{dd}", tag=f"c{li}_{dd}x1")
                    nc.any.memzero(c0[:])
                    c_prev.append(c0)
                # s8 gets 2 zero pad columns (8:10) so 4-wide slices starting
                # at col 6 stay in bounds; zero once per layer per dir.
                s8 = [work.tile([128, 10], dt.float32, name=f"s8_{dd}", tag=f"s{li}_{dd}")
                      for dd in range(NDIR)]
                for dd in range(NDIR):
                    nc.any.memzero(s8[dd][:])

                with tc.tile_pool(name=f"psR{li}", bufs=3, space="PSUM") as ppr:
                    for tt in range(t_len):
                        tix = (tt, t_len - 1 - tt)
                        rdc = (tt, t_len + 1 - tt)
                        wrc = (tt + 1, t_len - tt)
                        # both pre injections first, so neither direction's
                        # matmuls queue behind the other's chain in the DVE FIFO
                        pss = []
                        for dd in range(NDIR):
                            ps = ppr.tile([128, 8], dt.float32, name=f"ps{dd}", tag=f"ps{dd}")
                            pss.append(ps)
                            if tt >= 3:
                                nc.vector.tensor_copy(ps[:], pre[dd][:, :, tix[dd]])
                        for dd in range(NDIR):
                            ps = pss[dd]
                            rd_col, wr_col = rdc[dd], wrc[dd]
                            rhs0 = h8[dd][:, rd_col * 4:rd_col * 4 + 1]
                            rhs1 = h8[dd][:, rd_col * 4 + 1:rd_col * 4 + 2]
                            if tt < 3:
                                # prime this PSUM buffer's has_written bits
                                # with a real accumulation group
                                nc.tensor.matmul(ps[:], identb[:], pre[dd][:, :, tix[dd]],
                                                 start=True, stop=False)
                                for m in range(8):
                                    nc.tensor.matmul(ps[:, m:m + 1],
                                                     whh[dd][:, 0, m * 128:(m + 1) * 128],
                                                     rhs0, start=False, stop=False)
                                    nc.tensor.matmul(ps[:, m:m + 1],
                                                     whh[dd][:, 1, m * 128:(m + 1) * 128],
                                                     rhs1, start=False, stop=(m == 7))
                            else:
                                # matmuls accumulate onto the DVE-written pre
                                # (bank bits stay "written" since priming)
                                for m in range(8):
                                    nc.tensor.matmul(ps[:, m:m + 1],
                                                     whh[dd][:, 0, m * 128:(m + 1) * 128],
                                                     rhs0, start=False, stop=False,
                                                     skip_group_check=True)
                                    nc.tensor.matmul(ps[:, m:m + 1],
                                                     whh[dd][:, 1, m * 128:(m + 1) * 128],
                                                     rhs1, start=False, stop=False,
                                                     skip_group_check=True)
                            # s8[0:8] = sigmoid of the 8 gate chunks (x256 scale,
                            # g chunks additionally x2); cols 8:10 stay zero pad.
                            # Chain ops below run 4-wide (cols 2:4 of each are
                            # junk lanes) so the h8 store is one aligned 4B write.
                            nc.scalar.activation(s8[dd][:, 0:8], ps[:], AF.Sigmoid,
                                                 scale=0.00390625)
                            sd = s8[dd]
                            t1 = work.tile([128, 4], dt.float32, name=f"t1{dd}", tag=f"t1{li}_{dd}")
                            u = work.tile([128, 4], dt.float32, name=f"u{dd}", tag=f"u{li}_{dd}")
                            v = work.tile([128, 4], dt.float32, name=f"v{dd}", tag=f"v{li}_{dd}")
                            c_new = work.tile([128, 4], dt.float32, name=f"cn{dd}",
                                              tag=f"c{li}_{dd}x{tt % 2}")
                            # v first so c's operands are both ready when it
                            # reaches the DVE FIFO head
                            nc.vector.scalar_tensor_tensor(v[:], c_prev[dd][:], 0.0, sd[:, 2:6],
                                                           ALU.add, ALU.mult)
                            # t1 = 2*sg*si ; u = t1 - si   (tanh(g)=2*sigmoid(2g)-1)
                            nc.vector.scalar_tensor_tensor(t1[:], sd[:, 6:10], 2.0, sd[:, 0:4],
                                                           ALU.mult, ALU.mult)
                            nc.vector.scalar_tensor_tensor(u[:], t1[:], 1.0, sd[:, 0:4],
                                                           ALU.mult, ALU.subtract)
                            nc.vector.scalar_tensor_tensor(c_new[:], u[:], 0.0, v[:],
                                                           ALU.add, ALU.add)
                            # h8 = 8 * tanh(c) * so
                            th = work.tile([128, 4], dt.float32, name=f"th{dd}", tag=f"th{li}_{dd}")
                            nc.scalar.activation(th[:], c_new[:], AF.Tanh)
                            nc.vector.scalar_tensor_tensor(h8[dd][:, wr_col * 4:wr_col * 4 + 4],
                                                           th[:], 8.0, sd[:, 4:8],
                                                           ALU.mult, ALU.mult)
                            c_prev[dd] = c_new
                return h8

            def h8_tiles(h8):
                # [fwd_lo, fwd_hi, bwd_lo, bwd_hi] over the valid time range
                a, b_ = 4, 4 + 4 * t_len
                return [h8[0][:, a:b_:4], h8[0][0:72, a + 1:b_:4],
                        h8[1][:, a:b_:4], h8[1][0:72, a + 1:b_:4]]

            h0 = lstm_layer(0, Gtiles, [100] * 16, 16, 100)
            mk = [128, 72, 128, 72]
            m0_tiles = h8_tiles(h0)
            h1 = lstm_layer(1, m0_tiles, mk, 4, 128)
            m1_tiles = h8_tiles(h1)
            h2 = lstm_layer(2, m1_tiles, mk, 4, 128)
            m2_tiles = h8_tiles(h2)

            # ---------- heads ----------
            # logits carry a x128 scale: G-part weights are x128 (bf16),
            # M-part weights are x16 (fp8e3) against the x8-scaled h8; the
            # exp undoes it with scale=1/128.
            wpb = {}
            for nm, shp in [("wp1g", [100, 16, 1]), ("wp1m", [128, 4, 1]),
                            ("wp2g", [100, 16, 1]), ("wp2m", [128, 4, 1])]:
                tl = work.tile(shp, dt.float32, name=nm, tag=nm)
                nc.sync.dma_start(out=tl[:], in_=d[nm].ap())
                wdt = dt.float8e3 if nm.endswith("m") else dt.bfloat16
                tb = work.tile(shp, wdt, name=nm + "b", tag=nm + "b")
                nc.any.tensor_copy(tb[:], tl[:])
                wpb[nm] = tb

            with tc.tile_pool(name="psH", bufs=2, space="PSUM") as pph:
                def head(gname, mname, m_tiles, out_name):
                    e_all = work.tile([128, NT], dt.float32, name=f"e_{gname}")
                    for tch2 in range(NT):
                        sl = slice(tch2 * 128, (tch2 + 1) * 128)
                        ps_l = pph.tile([128, 1], dt.float32, name="ps_l", tag="ps_l")
                        for gi in range(16):
                            nc.tensor.matmul(ps_l[:], Gtiles[gi][:, sl], wpb[gname][:, gi],
                                             start=(gi == 0), stop=False)
                        for ki in range(4):
                            nc.tensor.matmul(ps_l[:], m_tiles[ki][0:mk[ki], sl],
                                             wpb[mname][0:mk[ki], ki],
                                             start=False, stop=(ki == 3))
                        nc.scalar.activation(e_all[:, tch2:tch2 + 1], ps_l[:], AF.Exp,
                                             scale=0.0078125)
                    ps_z = pph.tile([1, NT], dt.float32, name="ps_z", tag="ps_z")
                    nc.tensor.matmul(ps_z[:], ones[:, 0:1], e_all[:], start=True, stop=True)
                    z = work.tile([1, 1], dt.float32, name=f"z_{gname}")
                    nc.vector.tensor_reduce(z[:], ps_z[:], mybir.AxisListType.X, ALU.add)
                    iz = work.tile([1, 1], dt.float32, name=f"iz_{gname}")
                    nc.vector.reciprocal(iz[:], z[:])
                    ps_i = pph.tile([128, 1], dt.float32, name="ps_i", tag="ps_i")
                    nc.tensor.matmul(ps_i[:], ones[0:1, :], iz[:], start=True, stop=True)
                    izr = work.tile([128, 1], dt.float32, name=f"izr_{gname}")
                    nc.any.tensor_copy(izr[:], ps_i[:])
                    pout = work.tile([128, NT], dt.float32, name=f"pout_{gname}")
                    nc.vector.tensor_scalar_mul(pout[:], e_all[:], izr[:])
                    nc.sync.dma_start(out=d[out_name].ap(), in_=pout[:])

                head("wp1g", "wp1m", m1_tiles, "p1out")
                head("wp2g", "wp2m", m2_tiles, "p2out")

    nc.compile()
    return nc


def _prep_shared(w_s, Wih0, Whh0, b0, Wih1, Whh1, b1, w_p1, Wih2, Whh2, b2, w_p2, b_p2):
    sh = {}
    w1v, w2v, w3v = w_s[:D2], w_s[D2:2 * D2], w_s[2 * D2:]
    sh["w1"] = w1v.reshape(4, 100).T.reshape(100, 4, 1).astype(np.float32).copy()
    sh["w3"] = w3v.reshape(4, 100).T.reshape(100, 4, 1).astype(np.float32).copy()
    sh["w2rep"] = np.repeat(w2v.reshape(4, 100).T.reshape(100, 4, 1), 128, axis=2).astype(np.float32)
    sh["ident"] = np.eye(128, dtype=np.float32)
    sh["ones"] = np.ones((128, 128), np.float32)

    # whh carries x32 (fits fp8e3 normal range), h8 carries x8, so the full
    # recurrent product is x256; wih/b are x256 so preactivations match, and
    # the scan's sigmoid uses scale=1/256 to undo it.
    QW, QP = 32.0, 256.0

    def whh_pack(Whh):
        WP = perm_pad(np.swapaxes(Whh, 1, 2))  # [2, 200, 1024]
        WP[..., 768:1024] *= 2.0
        out = np.zeros((128, NDIR, 2, 1024), np.float32)
        out[:, :, 0] = WP[:, 0:128].transpose(1, 0, 2)
        out[0:72, :, 1] = WP[:, 128:200].transpose(1, 0, 2)
        return np.clip(out * QW, -15.5, 15.5).astype(ml_dtypes.float8_e3m4)

    def bias_pack(b):
        bP = perm_pad(b)  # [2, 1024]
        bP[..., 768:1024] *= 2.0
        return (bP.reshape(NDIR, 8, 128).transpose(2, 0, 1) * QP).copy().astype(np.float32)

    def wih_pack(Wih, nk, kdim, fp8=False):
        WP = perm_pad(np.swapaxes(Wih, 1, 2))  # [2, IN, 1024]
        WP[..., 768:1024] *= 2.0
        out = np.zeros((kdim, NDIR, nk, 1024), np.float32)
        if kdim == 100:
            for k in range(nk):
                out[:, :, k] = WP[:, k * 100:(k + 1) * 100].transpose(1, 0, 2)
        else:
            for k, (a, b_) in enumerate(MBOUNDS):
                out[0:b_ - a, :, k] = WP[:, a:b_].transpose(1, 0, 2)
        if fp8:  # consumed against x8-scaled h8 -> x32 weight keeps x256 total
            return np.clip(out * QW, -15.5, 15.5).astype(ml_dtypes.float8_e3m4)
        return (out * QP).astype(BF)

    sh["wih0"] = wih_pack(Wih0, 16, 100)
    sh["wih1"] = wih_pack(Wih1, 4, 128, fp8=True)
    sh["wih2"] = wih_pack(Wih2, 4, 128, fp8=True)
    sh["whh0"] = whh_pack(Whh0)
    sh["whh1"] = whh_pack(Whh1)
    sh["whh2"] = whh_pack(Whh2)
    sh["b0"] = bias_pack(b0)
    sh["b1"] = bias_pack(b1)
    sh["b2"] = bias_pack(b2)

    def mpack(wm):
        out = np.zeros((128, 4, 1), np.float32)
        for k, (a, b_) in enumerate(MBOUNDS):
            out[0:b_ - a, k, 0] = wm[a:b_]
        return out

    # head logit scale: G-part x128, M-part x16 (reads x8-scaled h8)
    sh["wp1g"] = (w_p1[:1600].reshape(16, 100).T.reshape(100, 16, 1) * 128.0).astype(np.float32).copy()
    sh["wp1m"] = np.clip(mpack(w_p1[1600:]) * 16.0, -15.5, 15.5)
    sh["wp2g"] = (w_p2[:1600].reshape(16, 100).T.reshape(100, 16, 1) * 128.0).astype(np.float32).copy()
    sh["wp2m"] = np.clip(mpack(w_p2[1600:]) * 16.0, -15.5, 15.5)
    return sh


def _ensure_ntff_hook():
    """Dev-loop only: register the axon NTFF profile hook if the image's
    antenv lacks axon_hooks (concourse crashes on the import otherwise)."""
    try:
        from antenv.axon_hooks import get_axon_ntff_profile_hook  # noqa: F401
        return
    except ImportError:
        pass
    import types
    mod = types.ModuleType("antenv.axon_hooks")
    _hook = [None]
    mod.set_axon_ntff_profile_hook = lambda h: _hook.__setitem__(0, h)
    mod.get_axon_ntff_profile_hook = lambda: _hook[0]
    sys.modules["antenv.axon_hooks"] = mod
    try:
        import antenv
        antenv.axon_hooks = mod
    except ImportError:
        pass
    try:
        from trn_agent_boot.trn_boot import _ntff_profile_via_ctypes
        mod.set_axon_ntff_profile_hook(
            _ntff_profile_via_ctypes("/opt/axon/libaxon_pjrt.so"))
    except Exception as e:  # degrade: run untraced rather than crash
        print(f"ntff hook setup failed ({e}); running without trace")


def kernel(ec, eq, w_s, Wih0, Whh0, b0, Wih1, Whh1, b1, w_p1,
           Wih2, Whh2, b2, w_p2, b_p2, _t_len=T, _trace=False):
    if _trace:
        _ensure_ntff_hook()
    ec = np.asarray(ec, np.float32)
    eq = np.asarray(eq, np.float32)
    sh = _prep_shared(np.asarray(w_s), np.asarray(Wih0), np.asarray(Whh0), np.asarray(b0),
                      np.asarray(Wih1), np.asarray(Whh1), np.asarray(b1), np.asarray(w_p1),
                      np.asarray(Wih2), np.asarray(Whh2), np.asarray(b2), np.asarray(w_p2),
                      np.asarray(b_p2))
    if _t_len not in _CACHED:
        _CACHED[_t_len] = _build(_t_len)
    nc = _CACHED[_t_len]
    NT = _t_len // 128

    in_maps = []
    for b in range(B):
        im = dict(sh)
        ecb_ = ec[b, :_t_len]  # [T, 400]
        eqb_ = eq[b]
        im["ecT"] = ecb_.T.reshape(4, 100, _t_len).transpose(1, 0, 2).copy()
        im["ec"] = ecb_.reshape(NT, 128, D2).transpose(1, 0, 2).copy()
        im["eqT"] = eqb_.T.reshape(4, 100, J).transpose(1, 0, 2).copy()
        im["eq"] = eqb_.copy()
        in_maps.append(im)

    res = run_bass_kernel_spmd(nc, in_maps, list(range(B)), trace=_trace)
    kernel.last_exec_ns = res.exec_time_ns
    kernel.last_result = res
    p1 = np.zeros((B, _t_len), np.float32)
    p2 = np.zeros((B, _t_len), np.float32)
    for b in range(B):
        p1[b] = res.results[b]["p1out"][:, :NT].T.reshape(-1)
        p2[b] = res.results[b]["p2out"][:, :NT].T.reshape(-1)
    return (p1, p2)

